# revision 3
# baseline (speedup 1.0000x reference)
"""Trainium2 Bass kernel for nn_DecoderRNN (GRU decoder + vocab projection + log_softmax).

Strategy (8 NeuronCores):
 - GRU recurrence: hidden dim sharded 8 ways (each core computes 128 rows of each
   of the r/z/n gates); per-step AllGather (bf16) rebuilds the full hidden state.
 - gi = relu(emb[idx]) @ w_ih.T precomputed for all timesteps (batched matmul).
 - Output projection: vocab sharded 8 ways (4000 columns per core), bf16 matmul
   with f32 PSUM accumulation; sharded log_softmax via AllGather of per-core
   exp-sums (logits are tiny so no max-shift is needed for stability).

All matmul inputs are bf16 (f32 accumulation); gate math and softmax are f32.
"""
import sys

sys.path.insert(0, "/opt/trn_rl_repo")

from contextlib import ExitStack

import numpy as np
import ml_dtypes

import concourse.bass as bass
import concourse.mybir as mybir
import concourse.tile as tile
from concourse import bacc
from concourse.bass import IndirectOffsetOnAxis
from concourse.bass_utils import run_bass_kernel_spmd
from concourse.masks import make_identity

BF = ml_dtypes.bfloat16
AF = mybir.ActivationFunctionType
ALU = mybir.AluOpType

NCORES = 8
B, T, E, H, V = 64, 32, 512, 1024, 32000
BT = B * T            # 2048
VS = V // NCORES      # 4000 vocab cols per core
NB = 8                # vocab chunks per core
VC = VS // NB         # 500 cols per chunk
MT = BT // 128        # 16 row blocks
KH = H // 128         # 8 k-tiles over H
KE = E // 128         # 4 k-tiles over E
SOS = 0

_CACHED_NC = None


def _build_nc():
    nc = bacc.Bacc("TRN2", target_bir_lowering=False, debug=False, num_devices=NCORES)
    dt = mybir.dt

    emb_d = nc.dram_tensor("emb", [V, E], dt.float32, kind="ExternalInput").ap()
    idx_d = nc.dram_tensor("idx", [128, 16], dt.int32, kind="ExternalInput").ap()
    wih_d = nc.dram_tensor("wih", [128, KE, 384], dt.bfloat16, kind="ExternalInput").ap()
    whh_d = nc.dram_tensor("whh", [128, KH, 384], dt.bfloat16, kind="ExternalInput").ap()
    bias_d = nc.dram_tensor("bias4", [128, 4], dt.float32, kind="ExternalInput").ap()
    h0bf_d = nc.dram_tensor("h0bf", [128, KH, B], dt.bfloat16, kind="ExternalInput").ap()
    h0own_d = nc.dram_tensor("h0own", [128, B], dt.float32, kind="ExternalInput").ap()
    wout_d = nc.dram_tensor("wout", [128, KH, VS], dt.bfloat16, kind="ExternalInput").ap()
    bout_d = nc.dram_tensor("bout", [1, VS], dt.float32, kind="ExternalInput").ap()

    olp_d = nc.dram_tensor("out_lp", [BT, VS], dt.float32, kind="ExternalOutput").ap()
    oht_d = nc.dram_tensor("out_hT", [128, B], dt.float32, kind="ExternalOutput").ap()

    with ExitStack() as ctx:
        tc = ctx.enter_context(tile.TileContext(nc))
        cst = ctx.enter_context(tc.tile_pool(name="cst", bufs=1))

        idx_sb = cst.tile([128, 16], dt.int32)
        wih_sb = cst.tile([128, KE, 384], dt.bfloat16)
        whh_sb = cst.tile([128, KH, 384], dt.bfloat16)
        bias_sb = cst.tile([128, 4], dt.float32)
        h0bf_sb = cst.tile([128, KH, B], dt.bfloat16)
        h0own_sb = cst.tile([128, B], dt.float32)
        wout_sb = cst.tile([128, KH, VS], dt.bfloat16)
        bout_sb = cst.tile([1, VS], dt.float32)
        ident = cst.tile([128, 128], dt.float32)
        ones_sb = cst.tile([1, 128], dt.float32)
        bb_sb = cst.tile([128, VS], dt.float32)       # b_out broadcast to 128 rows
        hsT = cst.tile([128, KH, BT], dt.bfloat16)    # gathered hidden states (bf16)
        giT = cst.tile([128, 3, BT], dt.float32)      # input-gate preactivations

        nc.sync.dma_start(idx_sb[:], idx_d[:])
        nc.sync.dma_start(wih_sb[:], wih_d[:])
        nc.sync.dma_start(whh_sb[:], whh_d[:])
        nc.sync.dma_start(bias_sb[:], bias_d[:])
        nc.sync.dma_start(h0bf_sb[:], h0bf_d[:])
        nc.sync.dma_start(h0own_sb[:], h0own_d[:])
        nc.sync.dma_start(wout_sb[:], wout_d[:])
        nc.sync.dma_start(bout_sb[:], bout_d[:])
        make_identity(nc, ident[:])
        nc.gpsimd.memset(ones_sb[:], 1.0)

        # ---- b_out broadcast across partitions via K=1 matmul of ones ----
        with tc.tile_pool(name="psB", bufs=2, space="PSUM") as psB:
            for nb in range(NB):
                pb = psB.tile([128, VC], dt.float32, space="PSUM", tag="pb")
                nc.tensor.matmul(pb[:], lhsT=ones_sb[:1, :], rhs=bout_sb[:1, bass.ts(nb, VC)],
                                 start=True, stop=True)
                nc.vector.tensor_copy(bb_sb[:, bass.ts(nb, VC)], pb[:])

        # ---- embedding gather + transpose + relu (to bf16 x^T) ----
        with ExitStack() as sctx:
            gxp = sctx.enter_context(tc.tile_pool(name="gx", bufs=3))
            xtp = sctx.enter_context(tc.tile_pool(name="xt", bufs=1))
            psT = sctx.enter_context(tc.tile_pool(name="psT", bufs=3, space="PSUM"))
            psG = sctx.enter_context(tc.tile_pool(name="psG", bufs=2, space="PSUM"))
            xT = xtp.tile([128, KE, BT], dt.bfloat16)
            for j in range(16):
                gx = gxp.tile([128, E], dt.float32, tag="gx")
                nc.gpsimd.indirect_dma_start(
                    out=gx[:], out_offset=None, in_=emb_d[:],
                    in_offset=IndirectOffsetOnAxis(ap=idx_sb[:, j:j + 1], axis=0),
                )
                for e in range(KE):
                    pt = psT.tile([128, 128], dt.float32, space="PSUM", tag="pt")
                    nc.tensor.transpose(pt[:], gx[:, bass.ts(e, 128)], ident[:])
                    nc.scalar.activation(xT[:, e, bass.ts(j, 128)], pt[:], AF.Relu)

            # ---- gi^T = w_ih_slice @ x^T (+ folded biases) ----
            for m in range(3):
                for q in range(4):
                    pg = psG.tile([128, 512], dt.float32, space="PSUM", tag="pg")
                    for k in range(KE):
                        nc.tensor.matmul(pg[:], lhsT=wih_sb[:, k, bass.ts(m, 128)],
                                         rhs=xT[:, k, bass.ts(q, 512)],
                                         start=(k == 0), stop=(k == KE - 1))
                    nc.scalar.activation(giT[:, m, bass.ts(q, 512)], pg[:], AF.Identity,
                                         bias=bias_sb[:, m:m + 1])

        # ---- pools for GRU + projection ----
        psD = ctx.enter_context(tc.tile_pool(name="psD", bufs=1, space="PSUM"))
        psP = ctx.enter_context(tc.tile_pool(name="psP", bufs=2, space="PSUM"))
        gp = ctx.enter_context(tc.tile_pool(name="gates", bufs=2))
        hb = ctx.enter_context(tc.tile_pool(name="hbuf", bufs=2))
        ringp = ctx.enter_context(tc.tile_pool(name="ring", bufs=2))
        ethp = ctx.enter_context(tc.tile_pool(name="eth", bufs=2))
        outp = ctx.enter_context(tc.tile_pool(name="outsb", bufs=3))
        stp = ctx.enter_context(tc.tile_pool(name="stats", bufs=2))
        ccp = ctx.enter_context(tc.tile_pool(name="cc", bufs=3, space="DRAM"))

        rg = [list(range(NCORES))]

        def proj_mt(mt):
            ring = ringp.tile([128, NB, VC], dt.bfloat16, tag="ring")
            csums = stp.tile([128, NB], dt.float32, tag="csums")
            for nb in range(NB):
                pp = psP.tile([128, VC], dt.float32, space="PSUM", tag="pp")
                for k in range(KH):
                    nc.tensor.matmul(pp[:], lhsT=hsT[:, k, bass.ts(mt, 128)],
                                     rhs=wout_sb[:, k, bass.ts(nb, VC)],
                                     start=(k == 0), stop=(k == KH - 1))
                nc.vector.tensor_tensor(out=ring[:, nb, :], in0=pp[:],
                                        in1=bb_sb[:, bass.ts(nb, VC)], op=ALU.add)
                eth = ethp.tile([128, VC], dt.bfloat16, tag="eth")
                nc.scalar.activation(eth[:], ring[:, nb, :], AF.Exp,
                                     accum_out=csums[:, nb:nb + 1])
            sloc = stp.tile([128, 1], dt.float32, tag="sloc")
            nc.vector.tensor_reduce(sloc[:], csums[:], axis=mybir.AxisListType.X, op=ALU.add)
            csin = ccp.tile([128, 1], dt.float32, tag="csin")
            csout = ccp.tile([NCORES, 128, 1], dt.float32, tag="csout")
            nc.sync.dma_start(csin[:], sloc[:])
            nc.gpsimd.collective_compute("AllGather", ALU.bypass, replica_groups=rg,
                                         ins=[csin[:].opt()], outs=[csout[:].opt()])
            sall = stp.tile([128, NCORES], dt.float32, tag="sall")
            nc.sync.dma_start(sall[:], csout[:].rearrange("r p one -> p (r one)"))
            stot = stp.tile([128, 1], dt.float32, tag="stot")
            nc.vector.tensor_reduce(stot[:], sall[:], axis=mybir.AxisListType.X, op=ALU.add)
            lns = stp.tile([128, 1], dt.float32, tag="lns")
            nc.scalar.activation(lns[:], stot[:], AF.Ln)
            nls = stp.tile([128, 1], dt.float32, tag="nls")
            nc.vector.tensor_scalar_mul(nls[:], lns[:], -1.0)
            for nb in range(NB):
                ob = outp.tile([128, VC], dt.float32, tag="ob")
                nc.scalar.activation(ob[:], ring[:, nb, :], AF.Identity, bias=nls[:])
                nc.sync.dma_start(olp_d[bass.ts(mt, 128), bass.ts(nb, VC)], ob[:])

        hprev_own = h0own_sb
        prev_rhs = h0bf_sb
        for t in range(T):
            tc_cols = bass.ts(t, B)
            pss = []
            for m in range(3):
                ps = psD.tile([128, B], dt.float32, space="PSUM", tag=f"g{m}")
                for k in range(KH):
                    nc.tensor.matmul(ps[:], lhsT=whh_sb[:, k, bass.ts(m, 128)],
                                     rhs=prev_rhs[:, k, :],
                                     start=(k == 0), stop=(k == KH - 1))
                pss.append(ps)
            sr = gp.tile([128, B], dt.float32, tag="sr")
            nc.vector.tensor_tensor(out=sr[:], in0=pss[0][:], in1=giT[:, 0, tc_cols], op=ALU.add)
            rt = gp.tile([128, B], dt.float32, tag="rt")
            nc.scalar.activation(rt[:], sr[:], AF.Sigmoid)
            sz = gp.tile([128, B], dt.float32, tag="sz")
            nc.vector.tensor_tensor(out=sz[:], in0=pss[1][:], in1=giT[:, 1, tc_cols], op=ALU.add)
            zt = gp.tile([128, B], dt.float32, tag="zt")
            nc.scalar.activation(zt[:], sz[:], AF.Sigmoid)
            hn = gp.tile([128, B], dt.float32, tag="hn")
            nc.scalar.activation(hn[:], pss[2][:], AF.Identity, bias=bias_sb[:, 3:4])
            rn = gp.tile([128, B], dt.float32, tag="rn")
            nc.vector.tensor_tensor(out=rn[:], in0=rt[:], in1=hn[:], op=ALU.mult)
            pn = gp.tile([128, B], dt.float32, tag="pn")
            nc.vector.tensor_tensor(out=pn[:], in0=rn[:], in1=giT[:, 2, tc_cols], op=ALU.add)
            nn = gp.tile([128, B], dt.float32, tag="nn")
            nc.scalar.activation(nn[:], pn[:], AF.Tanh)
            dh = gp.tile([128, B], dt.float32, tag="dh")
            nc.vector.tensor_tensor(out=dh[:], in0=hprev_own[:], in1=nn[:], op=ALU.subtract)
            zh = gp.tile([128, B], dt.float32, tag="zh")
            nc.vector.tensor_tensor(out=zh[:], in0=zt[:], in1=dh[:], op=ALU.mult)
            hnew = hb.tile([128, B], dt.float32, tag="hnew")
            nc.vector.tensor_tensor(out=hnew[:], in0=nn[:], in1=zh[:], op=ALU.add)
            hnbf = hb.tile([128, B], dt.bfloat16, tag="hnbf")
            nc.vector.tensor_copy(hnbf[:], hnew[:])

            cin = ccp.tile([128, B], dt.bfloat16, tag="cin")
            cout = ccp.tile([NCORES, 128, B], dt.bfloat16, tag="cout")
            nc.sync.dma_start(cin[:], hnbf[:])
            nc.gpsimd.collective_compute("AllGather", ALU.bypass, replica_groups=rg,
                                         ins=[cin[:].opt()], outs=[cout[:].opt()])
            nc.sync.dma_start(hsT[:, :, tc_cols], cout[:].rearrange("r p b -> p r b"))

            if t == T - 1:
                nc.sync.dma_start(oht_d[:], hnew[:])
            hprev_own = hnew
            prev_rhs = hsT[:, :, tc_cols]

            if t % 2 == 1:
                proj_mt(t // 2)

    nc.compile()
    return nc


def _get_nc():
    global _CACHED_NC
    if _CACHED_NC is None:
        _CACHED_NC = _build_nc()
    return _CACHED_NC


def _prep_in_maps(encoder_outputs, encoder_hidden, target_tensor, emb,
                  w_ih, w_hh, b_ih, b_hh, w_out, b_out):
    emb = np.ascontiguousarray(np.asarray(emb, dtype=np.float32))
    enc_h = np.asarray(encoder_hidden, dtype=np.float32)
    tgt = np.asarray(target_tensor)
    w_ih = np.asarray(w_ih, dtype=np.float32)
    w_hh = np.asarray(w_hh, dtype=np.float32)
    b_ih = np.asarray(b_ih, dtype=np.float32)
    b_hh = np.asarray(b_hh, dtype=np.float32)
    w_out = np.asarray(w_out, dtype=np.float32)
    b_out = np.asarray(b_out, dtype=np.float32)

    shifted = np.concatenate(
        [np.full((B, 1), SOS, dtype=tgt.dtype), tgt[:, :-1]], axis=1)  # [B, T]
    idx_tm = shifted.T.reshape(-1).astype(np.int32)          # t-major [BT]
    idx_sb = np.ascontiguousarray(idx_tm.reshape(16, 128).T)  # [128, 16]

    h0T = enc_h[0].T.astype(np.float32)                       # [H, B]
    h0bf = np.ascontiguousarray(
        h0T.reshape(KH, 128, B).transpose(1, 0, 2)).astype(BF)  # [128, KH, B]

    bsum = b_ih + b_hh

    in_maps = []
    for c in range(NCORES):
        sl = slice(c * 128, (c + 1) * 128)
        rows = np.r_[c * 128:(c + 1) * 128,
                     H + c * 128:H + (c + 1) * 128,
                     2 * H + c * 128:2 * H + (c + 1) * 128]
        whh_s = w_hh[rows]                                    # [384, H]
        wih_s = w_ih[rows]                                    # [384, E]
        whh_t = np.ascontiguousarray(
            whh_s.T.reshape(KH, 128, 384).transpose(1, 0, 2)).astype(BF)
        wih_t = np.ascontiguousarray(
            wih_s.T.reshape(KE, 128, 384).transpose(1, 0, 2)).astype(BF)
        bias4 = np.stack([bsum[c * 128:(c + 1) * 128],
                          bsum[H + c * 128:H + (c + 1) * 128],
                          b_ih[2 * H + c * 128:2 * H + (c + 1) * 128],
                          b_hh[2 * H + c * 128:2 * H + (c + 1) * 128]],
                         axis=1).astype(np.float32)           # [128, 4]
        wout_s = w_out[c * VS:(c + 1) * VS]                   # [VS, H]
        wout_t = np.ascontiguousarray(
            wout_s.T.reshape(KH, 128, VS).transpose(1, 0, 2)).astype(BF)
        bout_s = np.ascontiguousarray(b_out[c * VS:(c + 1) * VS].reshape(1, VS))
        in_maps.append({
            "emb": emb,
            "idx": idx_sb,
            "wih": wih_t,
            "whh": whh_t,
            "bias4": np.ascontiguousarray(bias4),
            "h0bf": h0bf,
            "h0own": np.ascontiguousarray(h0T[sl]),
            "wout": wout_t,
            "bout": bout_s,
        })
    return in_maps


def _assemble(results):
    lp = np.concatenate([results[c]["out_lp"] for c in range(NCORES)], axis=1)  # [BT, V]
    lp = lp.reshape(T, B, V).transpose(1, 0, 2)               # [B, T, V]
    hT = np.concatenate([results[c]["out_hT"] for c in range(NCORES)], axis=0)  # [H, B]
    return np.ascontiguousarray(lp), np.ascontiguousarray(hT.T)[None]


def kernel(**inputs):
    in_maps = _prep_in_maps(**inputs)
    nc = _get_nc()
    res = run_bass_kernel_spmd(nc, in_maps, list(range(NCORES)))
    return _assemble(res.results)


if __name__ == "__main__":
    _get_nc()
    print("kernel built ok")
